# revision 5
# baseline (speedup 1.0000x reference)
"""Multi-head self-attention Trainium2 kernel (B=4, S=2048, D=1024, H=16, dk=64).

Sharding (8 cores): data-parallel over batch (4) x tensor-parallel over head
groups (2).  Core c handles batch c//2 and heads [8*(c%2), 8*(c%2)+8), i.e.
feature columns [512*(c%2), 512*(c%2)+512) of Wq/Wk/Wv (column split) and the
matching rows of Wo (row split).  Each core emits a partial [2048, 1024]
output; the host sums the two partials per batch and adds bo.

Device-side layout strategy (per core):
  - x^T (host-pretransposed, [1024, 2048]) streams in once.
  - Q^T, K^T computed feature-major [512, 2048] (f on partitions) so the
    scores matmul K_h Q_h^T needs no on-device transposes.  1/sqrt(dk) is
    folded into Wq/bq on the host.
  - V computed in natural [2048, 512] layout with a ones-column appended per
    head: the attention matmul V65^T @ E then yields both EV^T (64 rows) and
    the softmax denominator (row 64) in one accumulation.
  - scoresT = K_h Q_h^T is exponentiated directly (no max subtraction:
    scores are ~N(0,1) here, exp stays in fp32 range) on the ACT engine.
  - All fp32 matmuls are issued as float32r (fp22 truncation) which runs at
    full PE rate for moving dims >= 256.
"""

import numpy as np

import concourse.bass as bass
import concourse.mybir as mybir
import concourse.tile as tile
from concourse import bacc
from concourse.bass_utils import run_bass_kernel_spmd

F32 = mybir.dt.float32
BF16 = mybir.dt.bfloat16
F32R = mybir.dt.float32r

P = 128
D = 1024          # model dim
S = 2048          # sequence length
FH = 512          # local feature width (8 heads x 64)
H_LOC = 8         # heads per core
DK = 64           # head dim
N_DT = D // P     # 8 d-tiles
N_FT = FH // P    # 4 local feature tiles
N_ST = S // P     # 16 sequence tiles
N_SC = S // 512   # 4 sequence chunks of 512
QC = 512          # query chunk


def _emit(nc, tc, xT, wq, bq, wk, bk, wv, bv, wo, ones_d, out):
    Exp = mybir.ActivationFunctionType.Exp

    with tc.tile_pool(name="consts", bufs=1) as consts, \
         tc.tile_pool(name="persist", bufs=1) as persist:
        ones = consts.tile([1, 512], F32R)
        nc.sync.dma_start(out=ones, in_=ones_d[:, :])
        bq_sb = consts.tile([1, FH], F32R)
        nc.sync.dma_start(out=bq_sb, in_=bq[:, :])
        bk_sb = consts.tile([1, FH], F32R)
        nc.sync.dma_start(out=bk_sb, in_=bk[:, :])
        bv_sb = consts.tile([1, FH], F32R)
        nc.sync.dma_start(out=bv_sb, in_=bv[:, :])

        QT = persist.tile([P, N_FT, S], F32R, tag="QT")
        KT = persist.tile([P, N_FT, S], F32R, tag="KT")
        V65 = persist.tile([P, N_ST, H_LOC, DK + 1], BF16, tag="V65")
        nc.vector.memset(V65[:, :, :, DK:DK + 1], 1.0)

        # ---------------- Phase A: projections ----------------
        with tc.tile_pool(name="xt_pool", bufs=N_DT) as xt_pool, \
             tc.tile_pool(name="w_pool", bufs=2) as w_pool, \
             tc.tile_pool(name="psA", bufs=4, space="PSUM") as psA:
            xts = []
            for dt in range(N_DT):
                xt = xt_pool.tile([P, S], F32R, tag="xt", name=f"xt{dt}")
                nc.sync.dma_start(out=xt, in_=xT[dt * P:(dt + 1) * P, :])
                xts.append(xt)

            # Q^T and K^T, feature-major: psum[f, s] = sum_d W[d, f] x^T[d, s]
            for wd, bias_sb, dest, nm in ((wq, bq_sb, QT, "wqs"),
                                          (wk, bk_sb, KT, "wks")):
                w_sb = w_pool.tile([P, N_DT, FH], F32R, tag="w", name=nm)
                nc.sync.dma_start(
                    out=w_sb, in_=wd[:, :].rearrange("(dt p) f -> p dt f", p=P))
                for ft in range(N_FT):
                    for sc in range(N_SC):
                        ps = psA.tile([P, QC], F32, tag="psA", name="psqk")
                        for dt in range(N_DT):
                            nc.tensor.matmul(
                                ps,
                                (w_sb[:, dt, ft * P:(ft + 1) * P]),
                                (xts[dt][:, sc * QC:(sc + 1) * QC]),
                                start=(dt == 0), stop=False)
                        # bias via rank-1: bias[f] (x) ones[s]
                        nc.tensor.matmul(
                            ps, (bias_sb[:, ft * P:(ft + 1) * P]), (ones),
                            start=False, stop=True)
                        nc.vector.tensor_copy(
                            out=dest[:, ft, sc * QC:(sc + 1) * QC], in_=ps)

            # V natural: psum[s, f] = sum_d x^T[d, s] W[d, f]
            wv_sb = w_pool.tile([P, N_DT, FH], F32R, tag="w", name="wvs")
            nc.sync.dma_start(
                out=wv_sb, in_=wv[:, :].rearrange("(dt p) f -> p dt f", p=P))
            for st in range(N_ST):
                ps = psA.tile([P, FH], F32, tag="psA", name="psv")
                for dt in range(N_DT):
                    nc.tensor.matmul(
                        ps,
                        (xts[dt][:, st * P:(st + 1) * P]),
                        (wv_sb[:, dt, :]),
                        start=(dt == 0), stop=False)
                nc.tensor.matmul(
                    ps, (ones[:, 0:P]), (bv_sb), start=False, stop=True)
                nc.vector.tensor_copy(
                    out=V65[:, st, :, 0:DK],
                    in_=ps[:, :].rearrange("p (h d) -> p h d", h=H_LOC))

        # ---------------- Phase B: attention ----------------
        with tc.tile_pool(name="ao_pool", bufs=1) as ao_pool, \
             tc.tile_pool(name="wo_pool", bufs=1) as wo_pool:
            AO = ao_pool.tile([P, N_FT, S], F32R)
            wo_sb = wo_pool.tile([P, N_FT, D], F32R)
            nc.sync.dma_start(
                out=wo_sb, in_=wo[:, :].rearrange("(ft p) e -> p ft e", p=P))

            with tc.tile_pool(name="e_pool", bufs=4) as e_pool, \
                 tc.tile_pool(name="r_pool", bufs=4) as r_pool, \
                 tc.tile_pool(name="psSC", bufs=2, space="PSUM") as psSC, \
                 tc.tile_pool(name="psEV", bufs=4, space="PSUM") as psEV:
                for t in range(N_FT):
                    for qc in range(N_SC):
                        ev = [psEV.tile([DK + 1, QC], F32, tag="ev",
                                        name=f"ev{h2}") for h2 in range(2)]
                        for kt in range(N_ST):
                            # scoresT[j, i] for the head pair (2t, 2t+1):
                            # rows 0-63 of KT/QT tile t = head 2t, rows 64-127
                            # = head 2t+1 (row-paired on the PE array).
                            ps = psSC.tile([P, 2 * QC], F32, tag="sc",
                                           name="scps")
                            for h2 in range(2):
                                lo = h2 * DK
                                nc.tensor.matmul(
                                    ps[:, h2 * QC:(h2 + 1) * QC],
                                    (KT[lo:lo + DK, t, kt * P:(kt + 1) * P]),
                                    (QT[lo:lo + DK, t, qc * QC:(qc + 1) * QC]),
                                    start=True, stop=True,
                                    skip_group_check=True)
                            e = e_pool.tile([P, 2 * QC], BF16, tag="e",
                                            name="esb")
                            nc.scalar.activation(out=e, in_=ps, func=Exp)
                            for h2 in range(2):
                                nc.tensor.matmul(
                                    ev[h2],
                                    V65[:, kt, 2 * t + h2, :],
                                    e[:, h2 * QC:(h2 + 1) * QC],
                                    start=(kt == 0), stop=(kt == N_ST - 1),
                                    skip_group_check=True)
                        for h2 in range(2):
                            # normalize: AO^T[f, i] = EV^T[f, i] / d[i]
                            r1 = r_pool.tile([1, QC], F32, tag="r1", name="r1")
                            nc.vector.reciprocal(out=r1, in_=ev[h2][DK:DK + 1, :])
                            rb = r_pool.tile([DK, QC], F32, tag="rb", name="rb")
                            nc.gpsimd.partition_broadcast(rb, r1)
                            nc.vector.tensor_mul(
                                out=AO[h2 * DK:(h2 + 1) * DK, t,
                                       qc * QC:(qc + 1) * QC],
                                in0=ev[h2][0:DK, :], in1=rb)

            # ---------------- Phase C: output projection ----------------
            with tc.tile_pool(name="o_pool", bufs=4) as o_pool, \
                 tc.tile_pool(name="psC", bufs=4, space="PSUM") as psC:
                for st in range(N_ST):
                    for ec in range(D // QC):
                        ps = psC.tile([P, QC], F32, tag="po", name="pso")
                        for ft in range(N_FT):
                            nc.tensor.matmul(
                                ps,
                                (AO[:, ft, st * P:(st + 1) * P]),
                                (wo_sb[:, ft, ec * QC:(ec + 1) * QC]),
                                start=(ft == 0), stop=(ft == N_FT - 1))
                        ob = o_pool.tile([P, QC], F32, tag="ob", name="ob")
                        nc.vector.tensor_copy(out=ob, in_=ps)
                        nc.sync.dma_start(
                            out=out[st * P:(st + 1) * P, ec * QC:(ec + 1) * QC],
                            in_=ob)


def build_nc(debug=False):
    nc = bacc.Bacc("TRN2", debug=debug)
    xT = nc.declare_dram_parameter("xT", [D, S], F32R, isOutput=False)
    wq = nc.declare_dram_parameter("wq", [D, FH], F32R, isOutput=False)
    bq = nc.declare_dram_parameter("bq", [1, FH], F32R, isOutput=False)
    wk = nc.declare_dram_parameter("wk", [D, FH], F32R, isOutput=False)
    bk = nc.declare_dram_parameter("bk", [1, FH], F32R, isOutput=False)
    wv = nc.declare_dram_parameter("wv", [D, FH], F32R, isOutput=False)
    bv = nc.declare_dram_parameter("bv", [1, FH], F32R, isOutput=False)
    wo = nc.declare_dram_parameter("wo", [FH, D], F32R, isOutput=False)
    ones_d = nc.declare_dram_parameter("ones_d", [1, 512], F32R, isOutput=False)
    out = nc.declare_dram_parameter("out", [S, D], F32, isOutput=True)
    with tile.TileContext(nc) as tc:
        _emit(nc, tc, xT[:, :], wq[:, :], bq[:, :], wk[:, :], bk[:, :],
              wv[:, :], bv[:, :], wo[:, :], ones_d[:, :], out[:, :])
    nc.compile()
    return nc


def make_in_maps(x, Wq, bq, Wk, bk, Wv, bv, Wo):
    in_maps = []
    for c in range(8):
        b, hg = divmod(c, 2)
        F = slice(FH * hg, FH * (hg + 1))
        in_maps.append({
            "xT": np.ascontiguousarray(x[b].T),
            "wq": np.ascontiguousarray(Wq[:, F]) * 0.125,
            "bq": (bq[F] * 0.125).reshape(1, FH),
            "wk": np.ascontiguousarray(Wk[:, F]),
            "bk": np.ascontiguousarray(bk[F]).reshape(1, FH),
            "wv": np.ascontiguousarray(Wv[:, F]),
            "bv": np.ascontiguousarray(bv[F]).reshape(1, FH),
            "wo": np.ascontiguousarray(Wo[F, :]),
            "ones_d": np.ones((1, 512), np.float32),
        })
    return in_maps


_NC_CACHE = None


def _get_nc():
    global _NC_CACHE
    if _NC_CACHE is None:
        _NC_CACHE = build_nc()
    return _NC_CACHE


def kernel(x, Wq, bq, Wk, bk, Wv, bv, Wo, bo, _trace=False):
    x = np.asarray(x, np.float32)
    args = [np.asarray(a, np.float32) for a in (Wq, bq, Wk, bk, Wv, bv, Wo)]
    bo = np.asarray(bo, np.float32)
    nc = _get_nc()
    in_maps = make_in_maps(x, *args)
    res = run_bass_kernel_spmd(nc, in_maps, list(range(8)), trace=_trace)
    out = np.empty((4, S, D), np.float32)
    for b in range(4):
        out[b] = res.results[2 * b]["out"] + res.results[2 * b + 1]["out"] + bo
    if _trace:
        return out, res
    return out
